# revision 5
# baseline (speedup 1.0000x reference)
"""Multi-head quasi-LSTM cell on 8 Trainium2 NeuronCores.

Math: the reference's block/decay-matrix machinery is exactly the elementwise
linear recurrence  c_t = sigmoid(fx_t + 1) * c_{t-1} + sigmoid(ix_t)*tanh(zx_t)
per (batch, head, dim) lane, followed by h_t = sigmoid(ox_t) * tanh(c_t),
with gate pre-activations from four (B*S, HDIM) @ (HDIM, H*D) matmuls and a
final (B*S, H*D) @ (H*D, HDIM) projection (EPS terms in the reference perturb
results only at the ~1e-6 level).

Sharding: sequence-parallel. Core i handles timesteps [i*256, (i+1)*256) for
all batches/heads. The only cross-core dependency is the scan carry; each core
computes a zero-init local scan end + full-chunk decay product, a 32KB
AllGather shares them, every core redundantly combines the 8-chunk chain and
one-hot-selects its own true initial state, then re-scans exactly.

Layouts on device (SBUF partition dim = channel ch = h*64+d, tiled by 128):
  gate inputs are host-transposed to (HDIM, rows) with rows = b*256 + s_local
  so the contraction dim sits on partitions for the PE; gate outputs land as
  (ch, rows), which is simultaneously the scan layout (time along free axis)
  and the lhsT layout for the output projection.
"""

import numpy as np

import concourse.bass as bass
import concourse.tile as tile
from concourse import bacc, mybir
from concourse.bass_utils import run_bass_kernel_spmd

B, S, HDIM = 4, 2048, 1024
H, D = 16, 64
HD = H * D                 # 1024
N_CORES = 8
S_LOC = S // N_CORES       # 256
ROWS = B * S_LOC           # 1024
KT = HDIM // 128           # 8 contraction tiles
MT = HD // 128             # 8 channel tiles
RT = ROWS // 128           # 8 row tiles
NHALF = ROWS // 512        # 2 matmul free-dim blocks

fp32 = mybir.dt.float32
fp32r = mybir.dt.float32r
AF = mybir.ActivationFunctionType
OP = mybir.AluOpType
AX = mybir.AxisListType

_CACHE = {}


def _build():
    nc = bacc.Bacc("TRN2", target_bir_lowering=False, debug=False,
                   num_devices=N_CORES)

    dx = {g: nc.dram_tensor(f"x{g}T", [HDIM, ROWS], fp32r,
                            kind="ExternalInput").ap() for g in "izfo"}
    dw = {g: nc.dram_tensor(f"W{g}", [HDIM, HD], fp32r,
                            kind="ExternalInput").ap() for g in "izfo"}
    dwp = nc.dram_tensor("Wp", [HD, HDIM], fp32r, kind="ExternalInput").ap()
    dbias = {g: nc.dram_tensor(f"b{g}", [128, MT], fp32,
                               kind="ExternalInput").ap() for g in "izfo"}
    dbp = nc.dram_tensor("bpT", [1, HDIM], fp32r, kind="ExternalInput").ap()
    dones = nc.dram_tensor("onesr", [1, 128], fp32r, kind="ExternalInput").ap()
    dc0 = nc.dram_tensor("c0t", [128, MT * B], fp32, kind="ExternalInput").ap()
    dsel = nc.dram_tensor("selmask", [128, N_CORES], fp32,
                          kind="ExternalInput").ap()
    dy = nc.dram_tensor("y_out", [ROWS, HDIM], fp32, kind="ExternalOutput").ap()
    dcend = nc.dram_tensor("c_end", [128, MT * B], fp32,
                           kind="ExternalOutput").ap()
    dhend = nc.dram_tensor("h_end", [128, MT * B], fp32,
                           kind="ExternalOutput").ap()

    with tile.TileContext(nc) as tc:
        with tc.tile_pool(name="xs", bufs=9) as xpool, \
             tc.tile_pool(name="ws", bufs=9) as wpool, \
             tc.tile_pool(name="gps", bufs=4, space="PSUM") as gps, \
             tc.tile_pool(name="yps", bufs=4, space="PSUM") as yps, \
             tc.tile_pool(name="sb", bufs=1) as sb, \
             tc.tile_pool(name="tzp", bufs=2) as tzp, \
             tc.tile_pool(name="scp", bufs=2) as scp, \
             tc.tile_pool(name="osigp", bufs=4) as osigp, \
             tc.tile_pool(name="ccp", bufs=2) as ccp, \
             tc.tile_pool(name="ydp", bufs=2) as ydp, \
             tc.tile_pool(name="dram", bufs=1, space="DRAM") as dramp:

            bias_t = {}
            for g in "izfo":
                bias_t[g] = sb.tile([128, MT], fp32, tag=f"bias_{g}", name=f"bias_{g}")
                nc.sync.dma_start(bias_t[g][:], dbias[g][:])
            c0_t = sb.tile([128, MT * B], fp32, tag="c0t")
            nc.sync.dma_start(c0_t[:], dc0[:])
            sel_t = sb.tile([128, N_CORES], fp32, tag="sel")
            nc.sync.dma_start(sel_t[:], dsel[:])
            bp_t = sb.tile([1, HDIM], fp32r, tag="bpt")
            nc.sync.dma_start(bp_t[:], dbp[:])
            ones1 = sb.tile([1, 128], fp32r, tag="ones1")
            nc.sync.dma_start(ones1[:], dones[:])

            ends_t = sb.tile([128, 64], fp32, tag="ends")
            gath_t = sb.tile([128, N_CORES * 64], fp32, tag="gath")
            call_t = sb.tile([128, (N_CORES + 1) * MT * B], fp32, tag="call")
            cin_t = sb.tile([128, MT * B], fp32, tag="cin")
            cend_t = sb.tile([128, MT * B], fp32, tag="cendt")
            hend_t = sb.tile([128, MT * B], fp32, tag="hendt")

            F = sb.tile([128, MT * ROWS], fp32, tag="F")
            U = sb.tile([128, MT * ROWS], fp32r, tag="U")

            def load_strips(dram_ap, width, pool, tag):
                strips = []
                for k in range(KT):
                    t = pool.tile([128, width], fp32r, tag=tag)
                    nc.sync.dma_start(t[:], dram_ap[k * 128:(k + 1) * 128, :])
                    strips.append(t)
                return strips

            def gate_psums(g):
                """Yield (m, n, psum) for one gate's 16 output tiles."""
                xs = load_strips(dx[g], ROWS, xpool, "xstrip")
                ws = load_strips(dw[g], HD, wpool, "wstrip")
                for m in range(MT):
                    for n in range(NHALF):
                        ps = gps.tile([128, 512], fp32, tag="gps")
                        for k in range(KT):
                            nc.tensor.matmul(
                                ps[:],
                                ws[k][:, m * 128:(m + 1) * 128],
                                xs[k][:, n * 512:(n + 1) * 512],
                                start=(k == 0), stop=(k == KT - 1))
                        yield m, n, ps

            # ---- gate i: U = sigmoid(ix) ----
            for m, n, ps in gate_psums("i"):
                nc.scalar.activation(
                    U[:, m * ROWS + n * 512: m * ROWS + (n + 1) * 512],
                    ps[:], AF.Sigmoid, bias=bias_t["i"][:, m:m + 1])

            # ---- gate z: U *= tanh(zx) ----
            for m, n, ps in gate_psums("z"):
                tz = tzp.tile([128, 512], fp32, tag="tz")
                nc.scalar.activation(tz[:], ps[:], AF.Tanh,
                                     bias=bias_t["z"][:, m:m + 1])
                usl = U[:, m * ROWS + n * 512: m * ROWS + (n + 1) * 512]
                nc.vector.tensor_mul(usl, usl, tz[:])

            # ---- gate f: F = sigmoid(fx + 1); then local scans per m ----
            for m, n, ps in gate_psums("f"):
                nc.scalar.activation(
                    F[:, m * ROWS + n * 512: m * ROWS + (n + 1) * 512],
                    ps[:], AF.Sigmoid, bias=bias_t["f"][:, m:m + 1])
                if n == NHALF - 1:
                    scr = scp.tile([128, ROWS], fp32, tag="scr")
                    for b in range(B):
                        sl = slice(m * ROWS + b * S_LOC,
                                   m * ROWS + (b + 1) * S_LOC)
                        nc.vector.tensor_tensor_scan(
                            scr[:, b * S_LOC:(b + 1) * S_LOC],
                            F[:, sl], U[:, sl], 0.0, OP.mult, OP.add)
                    # chunk-end local c per b (strided cols), into ends[:, m*B:...]
                    scr_v = scr[:].rearrange("p (b s) -> p b s", b=B)
                    nc.vector.tensor_copy(
                        ends_t[:, m * B:(m + 1) * B],
                        scr_v[:, :, S_LOC - 1])
                    # full-chunk decay product per b
                    f_v = F[:, m * ROWS:(m + 1) * ROWS].rearrange(
                        "p (b s) -> p b s", b=B)
                    nc.vector.tensor_reduce(
                        ends_t[:, 32 + m * B: 32 + (m + 1) * B],
                        f_v, axis=AX.X, op=OP.mult)

            # ---- allgather chunk summaries (32KB) ----
            in_b = dramp.tile([128, 64], fp32, tag="agin")
            out_b = dramp.tile([128 * N_CORES, 64], fp32, tag="agout")
            nc.gpsimd.dma_start(in_b[:], ends_t[:])
            nc.gpsimd.collective_compute(
                "AllGather", OP.bypass,
                replica_groups=[list(range(N_CORES))],
                ins=[in_b.opt()], outs=[out_b.opt()])
            nc.gpsimd.dma_start(
                gath_t[:].rearrange("p (j q) -> p j q", j=N_CORES),
                out_b[:].rearrange("(j p) q -> p j q", p=128))

            # ---- gate o (keeps PE busy during the collective) ----
            osigs = {}
            for m, n, ps in gate_psums("o"):
                if n == 0:
                    osigs[m] = osigp.tile([128, ROWS], fp32, tag="osig", name=f"osig{m}")
                nc.scalar.activation(
                    osigs[m][:, n * 512:(n + 1) * 512],
                    ps[:], AF.Sigmoid, bias=bias_t["o"][:, m:m + 1])

            # ---- cross-chunk chain (identical on every core) + select ----
            nc.vector.tensor_copy(call_t[:, 0:32], c0_t[:])
            for j in range(N_CORES):
                ej = gath_t[:, j * 64: j * 64 + 32]
                pj = gath_t[:, j * 64 + 32: j * 64 + 64]
                tmp = sb.tile([128, 32], fp32, tag="chaintmp")
                nc.vector.tensor_mul(tmp[:], pj, call_t[:, j * 32:(j + 1) * 32])
                nc.vector.tensor_add(call_t[:, (j + 1) * 32:(j + 2) * 32],
                                     tmp[:], ej)
            nc.vector.memset(cin_t[:], 0.0)
            for j in range(N_CORES):
                nc.vector.scalar_tensor_tensor(
                    cin_t[:], call_t[:, j * 32:(j + 1) * 32],
                    sel_t[:, j:j + 1], cin_t[:], OP.mult, OP.add)

            # ---- exact scan with true initial, h = sig_o * tanh(c) ----
            for m in range(MT):
                cc = ccp.tile([128, ROWS], fp32, tag="cc")
                for b in range(B):
                    sl = slice(m * ROWS + b * S_LOC,
                               m * ROWS + (b + 1) * S_LOC)
                    nc.vector.tensor_tensor_scan(
                        cc[:, b * S_LOC:(b + 1) * S_LOC],
                        F[:, sl], U[:, sl],
                        cin_t[:, m * B + b: m * B + b + 1],
                        OP.mult, OP.add)
                cc_v = cc[:].rearrange("p (b s) -> p b s", b=B)
                nc.vector.tensor_copy(cend_t[:, m * B:(m + 1) * B],
                                      cc_v[:, :, S_LOC - 1])
                nc.scalar.activation(cc[:], cc[:], AF.Tanh)
                # h overwrites U's slot (U is dead for this m)
                hsl = U[:, m * ROWS:(m + 1) * ROWS]
                nc.vector.tensor_mul(hsl, osigs[m][:], cc[:])
                h_v = U[:, m * ROWS:(m + 1) * ROWS].rearrange(
                    "p (b s) -> p b s", b=B)
                nc.vector.tensor_copy(hend_t[:, m * B:(m + 1) * B],
                                      h_v[:, :, S_LOC - 1])

            # ---- output projection y = h @ Wp + bp ----
            wps = load_strips(dwp, HDIM, wpool, "wstrip")
            for mr in range(RT):
                yd = ydp.tile([128, HDIM], fp32, tag="yd")
                for n in range(NHALF):
                    ps = yps.tile([128, 512], fp32, tag="yps")
                    for k in range(MT):
                        nc.tensor.matmul(
                            ps[:],
                            U[:, k * ROWS + mr * 128:
                              k * ROWS + mr * 128 + 128],
                            wps[k][:, n * 512:(n + 1) * 512],
                            start=(k == 0), stop=False)
                    nc.tensor.matmul(ps[:], ones1[:],
                                     bp_t[:, n * 512:(n + 1) * 512],
                                     start=False, stop=True)
                    dst = yd[:, n * 512:(n + 1) * 512]
                    if n == 0:
                        nc.vector.tensor_copy(dst, ps[:])
                    else:
                        nc.scalar.copy(dst, ps[:])
                nc.sync.dma_start(dy[mr * 128:(mr + 1) * 128, :], yd[:])

            nc.sync.dma_start(dcend[:], cend_t[:])
            nc.sync.dma_start(dhend[:], hend_t[:])

    nc.compile()
    return nc


def _get_nc():
    if "nc" not in _CACHE:
        _CACHE["nc"] = _build()
    return _CACHE["nc"]


def _xt_chunks(x):
    """(B,S,HDIM) -> per-core (HDIM, ROWS) with rows = b*S_LOC + s."""
    xt = np.asarray(x, dtype=np.float32).transpose(2, 0, 1)  # (HDIM, B, S)
    return [np.ascontiguousarray(
        xt[:, :, i * S_LOC:(i + 1) * S_LOC]).reshape(HDIM, ROWS)
        for i in range(N_CORES)]


def _small_lanes(v):
    """(B,H,D) -> (128, MT*B) with [p, m*B+b] = v[b, ch] for ch = m*128+p."""
    a = np.asarray(v, dtype=np.float32).reshape(B, HD).T      # (HD, B)
    return np.ascontiguousarray(
        a.reshape(MT, 128, B).transpose(1, 0, 2).reshape(128, MT * B))


def _lanes_to_bhd(a):
    """Inverse of _small_lanes."""
    return np.ascontiguousarray(
        a.reshape(128, MT, B).transpose(1, 0, 2).reshape(HD, B).T
    ).reshape(B, H, D)


def _bias_cols(b):
    return np.ascontiguousarray(
        np.asarray(b, dtype=np.float32).reshape(MT, 128).T)


def kernel(f_in, i_in, z_in, o_in, c0, h0, Wf, bf, Wi, bi, Wz, bz, Wo, bo,
           Wp, bp, _run_kwargs=None):
    nc = _get_nc()

    xf = _xt_chunks(f_in)
    xi = _xt_chunks(i_in)
    xz = _xt_chunks(z_in)
    xo = _xt_chunks(o_in)

    weights = {
        "Wf": np.ascontiguousarray(np.asarray(Wf, np.float32)),
        "Wi": np.ascontiguousarray(np.asarray(Wi, np.float32)),
        "Wz": np.ascontiguousarray(np.asarray(Wz, np.float32)),
        "Wo": np.ascontiguousarray(np.asarray(Wo, np.float32)),
        "Wp": np.ascontiguousarray(np.asarray(Wp, np.float32)),
    }
    biases = {
        "bf": _bias_cols(np.asarray(bf, np.float32) + 1.0),
        "bi": _bias_cols(bi),
        "bz": _bias_cols(bz),
        "bo": _bias_cols(bo),
    }
    bpT = np.ascontiguousarray(np.asarray(bp, np.float32).reshape(1, HDIM))
    c0t = _small_lanes(c0)

    in_maps = []
    for i in range(N_CORES):
        sel = np.zeros((128, N_CORES), np.float32)
        sel[:, i] = 1.0
        in_maps.append({
            "xfT": xf[i], "xiT": xi[i], "xzT": xz[i], "xoT": xo[i],
            "Wf": weights["Wf"], "Wi": weights["Wi"], "Wz": weights["Wz"],
            "Wo": weights["Wo"], "Wp": weights["Wp"],
            "bf": biases["bf"], "bi": biases["bi"], "bz": biases["bz"],
            "bo": biases["bo"], "bpT": bpT, "c0t": c0t, "selmask": sel,
            "onesr": np.ones((1, 128), np.float32),
        })

    res = run_bass_kernel_spmd(nc, in_maps, core_ids=list(range(N_CORES)),
                               **(_run_kwargs or {}))
    if _run_kwargs:
        _CACHE["last_results"] = res

    y = np.concatenate(
        [res.results[i]["y_out"].reshape(B, S_LOC, HDIM)
         for i in range(N_CORES)], axis=1)
    last_c = _lanes_to_bhd(res.results[N_CORES - 1]["c_end"])
    last_h = _lanes_to_bhd(res.results[N_CORES - 1]["h_end"])
    return y, last_c, last_h


# revision 6
# speedup vs baseline: 1.6436x; 1.6436x over previous
"""Multi-head quasi-LSTM cell on 8 Trainium2 NeuronCores.

Math: the reference's block/decay-matrix machinery is exactly the elementwise
linear recurrence  c_t = sigmoid(fx_t + 1) * c_{t-1} + sigmoid(ix_t)*tanh(zx_t)
per (batch, head, dim) lane, followed by h_t = sigmoid(ox_t) * tanh(c_t),
with gate pre-activations from four (B*S, HDIM) @ (HDIM, H*D) matmuls and a
final (B*S, H*D) @ (H*D, HDIM) projection (EPS terms in the reference perturb
results only at the ~1e-6 level).

Sharding: sequence-parallel. Core i handles timesteps [i*256, (i+1)*256) for
all batches/heads. The only cross-core dependency is the scan carry; each core
computes a zero-init local scan end + full-chunk decay product, a 32KB
AllGather shares them, every core redundantly combines the 8-chunk chain and
one-hot-selects its own true initial state, then re-scans exactly.

Layouts on device (SBUF partition dim = channel ch = h*64+d, tiled by 128):
  gate inputs are host-transposed to (HDIM, rows) with rows = b*256 + s_local
  so the contraction dim sits on partitions for the PE; gate outputs land as
  (ch, rows), which is simultaneously the scan layout (time along free axis)
  and the lhsT layout for the output projection.
"""

import numpy as np

import concourse.bass as bass
import concourse.tile as tile
from concourse import bacc, mybir
from concourse.bass_utils import run_bass_kernel_spmd

B, S, HDIM = 4, 2048, 1024
H, D = 16, 64
HD = H * D                 # 1024
N_CORES = 8
S_LOC = S // N_CORES       # 256
ROWS = B * S_LOC           # 1024
KT = HDIM // 128           # 8 contraction tiles
MT = HD // 128             # 8 channel tiles
RT = ROWS // 128           # 8 row tiles
NHALF = ROWS // 512        # 2 matmul free-dim blocks

fp32 = mybir.dt.float32
fp32r = mybir.dt.float32r
AF = mybir.ActivationFunctionType
OP = mybir.AluOpType
AX = mybir.AxisListType

_CACHE = {}


def _build(num_devices=N_CORES, use_cc=True):
    nc = bacc.Bacc("TRN2", target_bir_lowering=False, debug=False,
                   num_devices=num_devices)

    dx = {g: nc.dram_tensor(f"x{g}T", [HDIM, ROWS], fp32r,
                            kind="ExternalInput").ap() for g in "izfo"}
    dw = {g: nc.dram_tensor(f"W{g}", [HDIM, HD], fp32r,
                            kind="ExternalInput").ap() for g in "izfo"}
    dwp = nc.dram_tensor("Wp", [HD, HDIM], fp32r, kind="ExternalInput").ap()
    dbias = {g: nc.dram_tensor(f"b{g}", [128, MT], fp32,
                               kind="ExternalInput").ap() for g in "izfo"}
    dbp = nc.dram_tensor("bpT", [1, HDIM], fp32r, kind="ExternalInput").ap()
    dones = nc.dram_tensor("onesr", [1, 128], fp32r, kind="ExternalInput").ap()
    dc0 = nc.dram_tensor("c0t", [128, MT * B], fp32, kind="ExternalInput").ap()
    dsel = nc.dram_tensor("selmask", [128, N_CORES], fp32,
                          kind="ExternalInput").ap()
    dy = nc.dram_tensor("y_out", [ROWS, HDIM], fp32, kind="ExternalOutput").ap()
    dcend = nc.dram_tensor("c_end", [128, MT * B], fp32,
                           kind="ExternalOutput").ap()
    dhend = nc.dram_tensor("h_end", [128, MT * B], fp32,
                           kind="ExternalOutput").ap()

    with tile.TileContext(nc) as tc:
        with tc.tile_pool(name="xs", bufs=9) as xpool, \
             tc.tile_pool(name="ws", bufs=9) as wpool, \
             tc.tile_pool(name="gps", bufs=4, space="PSUM") as gps, \
             tc.tile_pool(name="yps", bufs=4, space="PSUM") as yps, \
             tc.tile_pool(name="sb", bufs=1) as sb, \
             tc.tile_pool(name="tzp", bufs=2) as tzp, \
             tc.tile_pool(name="scp", bufs=2) as scp, \
             tc.tile_pool(name="osigp", bufs=4) as osigp, \
             tc.tile_pool(name="ccp", bufs=2) as ccp, \
             tc.tile_pool(name="ydp", bufs=2) as ydp, \
             tc.tile_pool(name="dram", bufs=1, space="DRAM") as dramp:

            bias_t = {}
            for g in "izfo":
                bias_t[g] = sb.tile([128, MT], fp32, tag=f"bias_{g}", name=f"bias_{g}")
                nc.sync.dma_start(bias_t[g][:], dbias[g][:])
            c0_t = sb.tile([128, MT * B], fp32, tag="c0t")
            nc.sync.dma_start(c0_t[:], dc0[:])
            sel_t = sb.tile([128, N_CORES], fp32, tag="sel")
            nc.sync.dma_start(sel_t[:], dsel[:])
            bp_t = sb.tile([1, HDIM], fp32r, tag="bpt")
            nc.sync.dma_start(bp_t[:], dbp[:])
            ones1 = sb.tile([1, 128], fp32r, tag="ones1")
            nc.sync.dma_start(ones1[:], dones[:])

            ends_t = sb.tile([128, 64], fp32, tag="ends")
            gath_t = sb.tile([128, N_CORES * 64], fp32, tag="gath")
            call_t = sb.tile([128, (N_CORES + 1) * MT * B], fp32, tag="call")
            cin_t = sb.tile([128, MT * B], fp32, tag="cin")
            cend_t = sb.tile([128, MT * B], fp32, tag="cendt")
            hend_t = sb.tile([128, MT * B], fp32, tag="hendt")

            F = sb.tile([128, MT * ROWS], fp32, tag="F")
            U = sb.tile([128, MT * ROWS], fp32r, tag="U")

            def load_strips(dram_ap, width, pool, tag):
                strips = []
                for k in range(KT):
                    t = pool.tile([128, width], fp32r, tag=tag)
                    nc.sync.dma_start(t[:], dram_ap[k * 128:(k + 1) * 128, :])
                    strips.append(t)
                return strips

            def gate_psums(g):
                """Yield (m, n, psum) for one gate's 16 output tiles."""
                xs = load_strips(dx[g], ROWS, xpool, "xstrip")
                ws = load_strips(dw[g], HD, wpool, "wstrip")
                for m in range(MT):
                    for n in range(NHALF):
                        ps = gps.tile([128, 512], fp32, tag="gps")
                        for k in range(KT):
                            nc.tensor.matmul(
                                ps[:],
                                ws[k][:, m * 128:(m + 1) * 128],
                                xs[k][:, n * 512:(n + 1) * 512],
                                start=(k == 0), stop=(k == KT - 1))
                        yield m, n, ps

            # ---- gate i: U = sigmoid(ix) ----
            for m, n, ps in gate_psums("i"):
                nc.scalar.activation(
                    U[:, m * ROWS + n * 512: m * ROWS + (n + 1) * 512],
                    ps[:], AF.Sigmoid, bias=bias_t["i"][:, m:m + 1])

            # ---- gate z: U *= tanh(zx) ----
            for m, n, ps in gate_psums("z"):
                tz = tzp.tile([128, 512], fp32, tag="tz")
                nc.scalar.activation(tz[:], ps[:], AF.Tanh,
                                     bias=bias_t["z"][:, m:m + 1])
                usl = U[:, m * ROWS + n * 512: m * ROWS + (n + 1) * 512]
                nc.vector.tensor_mul(usl, usl, tz[:])

            # ---- gate f: F = sigmoid(fx + 1); then local scans per m ----
            for m, n, ps in gate_psums("f"):
                nc.scalar.activation(
                    F[:, m * ROWS + n * 512: m * ROWS + (n + 1) * 512],
                    ps[:], AF.Sigmoid, bias=bias_t["f"][:, m:m + 1])
                if n == NHALF - 1:
                    scr = scp.tile([128, ROWS], fp32, tag="scr")
                    for b in range(B):
                        sl = slice(m * ROWS + b * S_LOC,
                                   m * ROWS + (b + 1) * S_LOC)
                        nc.vector.tensor_tensor_scan(
                            scr[:, b * S_LOC:(b + 1) * S_LOC],
                            F[:, sl], U[:, sl], 0.0, OP.mult, OP.add)
                    # chunk-end local c per b (strided cols), into ends[:, m*B:...]
                    scr_v = scr[:].rearrange("p (b s) -> p b s", b=B)
                    nc.vector.tensor_copy(
                        ends_t[:, m * B:(m + 1) * B],
                        scr_v[:, :, S_LOC - 1])
                    # full-chunk decay product per b
                    f_v = F[:, m * ROWS:(m + 1) * ROWS].rearrange(
                        "p (b s) -> p b s", b=B)
                    nc.vector.tensor_reduce(
                        ends_t[:, 32 + m * B: 32 + (m + 1) * B],
                        f_v, axis=AX.X, op=OP.mult)

            # ---- allgather chunk summaries (32KB) ----
            in_b = dramp.tile([128, 64], fp32, tag="agin")
            out_b = dramp.tile([128 * N_CORES, 64], fp32, tag="agout")
            nc.gpsimd.dma_start(in_b[:], ends_t[:])
            if use_cc:
                nc.gpsimd.collective_compute(
                    "AllGather", OP.bypass,
                    replica_groups=[list(range(N_CORES))],
                    ins=[in_b.opt()], outs=[out_b.opt()])
            else:
                for _j in range(N_CORES):
                    nc.gpsimd.dma_start(out_b[_j * 128:(_j + 1) * 128, :],
                                        in_b[:])
            nc.gpsimd.dma_start(
                gath_t[:].rearrange("p (j q) -> p j q", j=N_CORES),
                out_b[:].rearrange("(j p) q -> p j q", p=128))

            # ---- gate o (keeps PE busy during the collective) ----
            osigs = {}
            for m, n, ps in gate_psums("o"):
                if n == 0:
                    osigs[m] = osigp.tile([128, ROWS], fp32, tag="osig", name=f"osig{m}")
                nc.scalar.activation(
                    osigs[m][:, n * 512:(n + 1) * 512],
                    ps[:], AF.Sigmoid, bias=bias_t["o"][:, m:m + 1])

            # ---- cross-chunk chain (identical on every core) + select ----
            nc.vector.tensor_copy(call_t[:, 0:32], c0_t[:])
            for j in range(N_CORES):
                ej = gath_t[:, j * 64: j * 64 + 32]
                pj = gath_t[:, j * 64 + 32: j * 64 + 64]
                tmp = sb.tile([128, 32], fp32, tag="chaintmp")
                nc.vector.tensor_mul(tmp[:], pj, call_t[:, j * 32:(j + 1) * 32])
                nc.vector.tensor_add(call_t[:, (j + 1) * 32:(j + 2) * 32],
                                     tmp[:], ej)
            nc.vector.memset(cin_t[:], 0.0)
            for j in range(N_CORES):
                nc.vector.scalar_tensor_tensor(
                    cin_t[:], call_t[:, j * 32:(j + 1) * 32],
                    sel_t[:, j:j + 1], cin_t[:], OP.mult, OP.add)

            # ---- exact scan with true initial, h = sig_o * tanh(c) ----
            for m in range(MT):
                cc = ccp.tile([128, ROWS], fp32, tag="cc")
                for b in range(B):
                    sl = slice(m * ROWS + b * S_LOC,
                               m * ROWS + (b + 1) * S_LOC)
                    nc.vector.tensor_tensor_scan(
                        cc[:, b * S_LOC:(b + 1) * S_LOC],
                        F[:, sl], U[:, sl],
                        cin_t[:, m * B + b: m * B + b + 1],
                        OP.mult, OP.add)
                cc_v = cc[:].rearrange("p (b s) -> p b s", b=B)
                nc.vector.tensor_copy(cend_t[:, m * B:(m + 1) * B],
                                      cc_v[:, :, S_LOC - 1])
                nc.scalar.activation(cc[:], cc[:], AF.Tanh)
                # h overwrites U's slot (U is dead for this m)
                hsl = U[:, m * ROWS:(m + 1) * ROWS]
                nc.vector.tensor_mul(hsl, osigs[m][:], cc[:])
                h_v = U[:, m * ROWS:(m + 1) * ROWS].rearrange(
                    "p (b s) -> p b s", b=B)
                nc.vector.tensor_copy(hend_t[:, m * B:(m + 1) * B],
                                      h_v[:, :, S_LOC - 1])

            # ---- output projection y = h @ Wp + bp ----
            wps = load_strips(dwp, HDIM, wpool, "wstrip")
            for mr in range(RT):
                yd = ydp.tile([128, HDIM], fp32, tag="yd")
                for n in range(NHALF):
                    ps = yps.tile([128, 512], fp32, tag="yps")
                    for k in range(MT):
                        nc.tensor.matmul(
                            ps[:],
                            U[:, k * ROWS + mr * 128:
                              k * ROWS + mr * 128 + 128],
                            wps[k][:, n * 512:(n + 1) * 512],
                            start=(k == 0), stop=False)
                    nc.tensor.matmul(ps[:], ones1[:],
                                     bp_t[:, n * 512:(n + 1) * 512],
                                     start=False, stop=True)
                    dst = yd[:, n * 512:(n + 1) * 512]
                    if n == 0:
                        nc.vector.tensor_copy(dst, ps[:])
                    else:
                        nc.scalar.copy(dst, ps[:])
                nc.sync.dma_start(dy[mr * 128:(mr + 1) * 128, :], yd[:])

            nc.sync.dma_start(dcend[:], cend_t[:])
            nc.sync.dma_start(dhend[:], hend_t[:])

    nc.compile()
    return nc


def _get_nc():
    if "nc" not in _CACHE:
        _CACHE["nc"] = _build()
    return _CACHE["nc"]


def _xt_chunks(x):
    """(B,S,HDIM) -> per-core (HDIM, ROWS) with rows = b*S_LOC + s."""
    xt = np.asarray(x, dtype=np.float32).transpose(2, 0, 1)  # (HDIM, B, S)
    return [np.ascontiguousarray(
        xt[:, :, i * S_LOC:(i + 1) * S_LOC]).reshape(HDIM, ROWS)
        for i in range(N_CORES)]


def _small_lanes(v):
    """(B,H,D) -> (128, MT*B) with [p, m*B+b] = v[b, ch] for ch = m*128+p."""
    a = np.asarray(v, dtype=np.float32).reshape(B, HD).T      # (HD, B)
    return np.ascontiguousarray(
        a.reshape(MT, 128, B).transpose(1, 0, 2).reshape(128, MT * B))


def _lanes_to_bhd(a):
    """Inverse of _small_lanes."""
    return np.ascontiguousarray(
        a.reshape(128, MT, B).transpose(1, 0, 2).reshape(HD, B).T
    ).reshape(B, H, D)


def _bias_cols(b):
    return np.ascontiguousarray(
        np.asarray(b, dtype=np.float32).reshape(MT, 128).T)


def kernel(f_in, i_in, z_in, o_in, c0, h0, Wf, bf, Wi, bi, Wz, bz, Wo, bo,
           Wp, bp, _run_kwargs=None):
    nc = _get_nc()

    xf = _xt_chunks(f_in)
    xi = _xt_chunks(i_in)
    xz = _xt_chunks(z_in)
    xo = _xt_chunks(o_in)

    weights = {
        "Wf": np.ascontiguousarray(np.asarray(Wf, np.float32)),
        "Wi": np.ascontiguousarray(np.asarray(Wi, np.float32)),
        "Wz": np.ascontiguousarray(np.asarray(Wz, np.float32)),
        "Wo": np.ascontiguousarray(np.asarray(Wo, np.float32)),
        "Wp": np.ascontiguousarray(np.asarray(Wp, np.float32)),
    }
    biases = {
        "bf": _bias_cols(np.asarray(bf, np.float32) + 1.0),
        "bi": _bias_cols(bi),
        "bz": _bias_cols(bz),
        "bo": _bias_cols(bo),
    }
    bpT = np.ascontiguousarray(np.asarray(bp, np.float32).reshape(1, HDIM))
    c0t = _small_lanes(c0)

    in_maps = []
    for i in range(N_CORES):
        sel = np.zeros((128, N_CORES), np.float32)
        sel[:, i] = 1.0
        in_maps.append({
            "xfT": xf[i], "xiT": xi[i], "xzT": xz[i], "xoT": xo[i],
            "Wf": weights["Wf"], "Wi": weights["Wi"], "Wz": weights["Wz"],
            "Wo": weights["Wo"], "Wp": weights["Wp"],
            "bf": biases["bf"], "bi": biases["bi"], "bz": biases["bz"],
            "bo": biases["bo"], "bpT": bpT, "c0t": c0t, "selmask": sel,
            "onesr": np.ones((1, 128), np.float32),
        })

    res = run_bass_kernel_spmd(nc, in_maps, core_ids=list(range(N_CORES)),
                               **(_run_kwargs or {}))
    if _run_kwargs:
        _CACHE["last_results"] = res

    y = np.concatenate(
        [res.results[i]["y_out"].reshape(B, S_LOC, HDIM)
         for i in range(N_CORES)], axis=1)
    last_c = _lanes_to_bhd(res.results[N_CORES - 1]["c_end"])
    last_h = _lanes_to_bhd(res.results[N_CORES - 1]["h_end"])
    return y, last_c, last_h
